# revision 44
# baseline (speedup 1.0000x reference)
"""Trainium2 Bass kernel for nn_ControlledConvEMAStabilizer.

Pipeline (per batch image, one NeuronCore each, batch-parallel over 8 cores):
  q = cat(backbone, z, mem_stab, mem_unstab)          # 160ch
  q = lrelu(conv3x3(q, w0) + b0)                      # -> 64ch
  q = lrelu(conv3x3(q, w1) + b1)                      # -> 64ch
  q = lrelu(conv3x3(q, w2) + b2)                      # -> 64ch
  head = conv3x3(q, w_last) + b_last                  # -> 288ch = 9 taps x 32ch
  eta  = softmax([head; 0]) over the 9+1 slots
  out  = sum_p unfold(mem_stab)[p] * eta[p] + eta[9] * z

Implementation notes:
  - Feature maps live in SBUF as zero-padded flat rows: image pixel (r,c) at
    column 129*(r+1)+1+c (row stride 129).  Every 3x3 tap is a pure column
    offset, so convs are PSUM-accumulated matmuls over shifted views.
  - K-stacking: each q tile is [128, NCOL]: partitions 0:64 = q, partitions
    64:128 = q shifted by +129.  A K=128 matmul applies two vertical taps.
  - Strip-PAIR column tiling: the M=64 convs process two 3-row strips
    concurrently as independent PE column-group chains (strip A -> psum[0:64]
    / array cols 0:64, strip B -> psum[64:128] / cols 64:128).
  - A dummy warm-up matmul burst runs at t=0 (overlapping the input DMA) to
    lift the PE HAM clock gate (cold 1.2 GHz -> warm 2.4 GHz) before conv0.
  - mu3 (3 vertically-shifted copies of mem_unstab) loads straight from DRAM
    with 3-tap strided APs - no serial SBUF->SBUF shift chain.
  - Conv evac is a single Scalar Lrelu activation (alpha=0.01) writing the
    strided q view directly; one mirror DMA per strip completes the K-stack.
    This keeps Vector free so the PE never stalls on psum evacuation.
  - conv_last + fusion processed in 2-strip blocks:
      * taps 0-7 head -> 2x M=128 psum chunks per strip (serial chains),
      * tap-8 head -> M=32 matmuls col-tiled to PE column groups 2 and 3
        (strip s / s+1) sharing one weight load -> concurrent,
      * softmax reduce: 3 matmul rounds with shared stationary weights
        4-way col-tiled into one psum bank: [den_s, num_s, den_s1, num_s1];
        the custom tail weight folds the softmax "+1" (ones rows) in.
  - Patch loads (mem_stab unfold) use 2-strip windows and stride-1 tap-group
    APs; z and exp/product tails assemble per strip into one [128, n] rhs.
"""

import numpy as np
from contextlib import ExitStack

import concourse.bacc as bacc
import concourse.tile as tile
from concourse import mybir
from concourse.ap import AP
from concourse.bass_utils import run_bass_kernel_spmd

F32 = mybir.dt.float32
BF16 = mybir.dt.bfloat16
ALU = mybir.AluOpType
ACTF = mybir.ActivationFunctionType

H = 128
ST = 129                      # padded row stride
NCOL = ST * 130 + 2           # 16772 sbuf cols
XCOL = NCOL                   # dram padded cols for xpad
MUCOL = NCOL + 2 * ST + 2     # mu dram padded cols (reads up to +258)
RPS = 3                       # rows per strip

# taps in fusion/unfold order p = 3*kh + kw -> offset 129*(kh-1) + (kw-1)
P_TAPS = [ST * (kh - 1) + (kw - 1) for kh in range(3) for kw in range(3)]

W128_OFF = dict(w0c1=0, w1P=576, w2P=768, wlP=960, eye=1824, tail=1856)
W128_COLS = 1920
W96_OFF = dict(w0c2=0, w1S=192, w2S=384, wlS=576)
W96_COLS = 1440


def _j0(r0):
    return ST * (r0 + 1) + 1


def _strips():
    out = []
    r0 = 0
    while r0 < H:
        nr = min(RPS, H - r0)
        out.append((r0, nr))
        r0 += nr
    return out


def _build_program(debug=False):
    nc = bacc.Bacc("TRN2", target_bir_lowering=False, debug=False)

    d_xpad = nc.dram_tensor("xpad", [128, XCOL], BF16, kind="ExternalInput")
    d_mupad = nc.dram_tensor("mupad", [32, MUCOL], BF16, kind="ExternalInput")
    d_w128 = nc.dram_tensor("w128", [128, W128_COLS], BF16, kind="ExternalInput")
    d_w96 = nc.dram_tensor("w96", [96, W96_COLS], BF16, kind="ExternalInput")
    d_b = nc.dram_tensor("bias", [128, 3], F32, kind="ExternalInput")
    d_blp = nc.dram_tensor("blp", [128, 3], F32, kind="ExternalInput")
    d_out = nc.dram_tensor("out", [32, H, H], F32, kind="ExternalOutput")
    if debug:
        d_q1 = nc.dram_tensor("dbg_q1", [128, NCOL], BF16, kind="ExternalOutput")
        d_q2 = nc.dram_tensor("dbg_q2", [128, NCOL], BF16, kind="ExternalOutput")
        d_q3 = nc.dram_tensor("dbg_q3", [128, NCOL], BF16, kind="ExternalOutput")
        d_pc3 = nc.dram_tensor("dbg_pc3", [128, 3 * ST], F32, kind="ExternalOutput")
        d_pnd = nc.dram_tensor("dbg_pnd", [128, 3 * ST], F32, kind="ExternalOutput")
        d_ea1 = nc.dram_tensor("dbg_ea1", [128, 3 * ST], BF16, kind="ExternalOutput")
        d_ta1 = nc.dram_tensor("dbg_ta1", [128, 3 * ST], BF16, kind="ExternalOutput")
    DBG_BLOCK = 1

    strips = _strips()
    pairs = []
    i = 0
    while i < len(strips):
        if i + 1 < len(strips):
            pairs.append((strips[i], strips[i + 1]))
            i += 2
        else:
            pairs.append((strips[i], None))
            i += 1

    with tile.TileContext(nc) as tc, ExitStack() as ctx:
        wp = ctx.enter_context(tc.tile_pool(name="wp", bufs=1))
        big = ctx.enter_context(tc.tile_pool(name="big", bufs=1))
        fu = ctx.enter_context(tc.tile_pool(name="fu", bufs=2))
        pp = ctx.enter_context(tc.tile_pool(name="pp", bufs=2, space="PSUM"))
        ph = ctx.enter_context(tc.tile_pool(name="ph", bufs=3, space="PSUM"))
        pcp = ctx.enter_context(tc.tile_pool(name="pcp", bufs=2, space="PSUM"))
        pnd = ctx.enter_context(tc.tile_pool(name="pnd", bufs=1, space="PSUM"))

        # ---- HAM warm-up: dense dummy matmul burst, no data deps, runs
        # while the input DMAs stream.  ~18 N=512 matmuls = enough sustained
        # PE busy to lift the clock gate to 2.4 GHz before conv0 begins. ----
        wz = wp.tile([128, 512], BF16)
        nc.gpsimd.memset(wz[:], 0.0)
        wps = pp.tile([128, 512], F32, tag="pA", name="wps")
        for i in range(16):
            nc.tensor.matmul(wps[:, 0:512], wz[:, 0:128], wz[:, 0:512],
                             start=(i == 0), stop=(i == 15))

        # ---- weights / constants: conv0's weights load first (small), the
        # conv1/2/last weights stream later behind the early input chunks ----
        w128 = wp.tile([128, W128_COLS], BF16)
        w96 = wp.tile([96, W96_COLS], BF16)
        bias = wp.tile([128, 3], F32)
        blp = wp.tile([128, 3], F32)
        nc.sync.dma_start(out=w128[:, 0:576], in_=d_w128.ap()[:, 0:576])
        nc.scalar.dma_start(out=w96[:, 0:192], in_=d_w96.ap()[:, 0:192])
        nc.gpsimd.dma_start(out=bias[:], in_=d_b.ap())
        nc.gpsimd.dma_start(out=blp[:], in_=d_blp.ap())

        def w128s(name, i, m0, mw, step=64):
            o = W128_OFF[name] + i * step + m0
            return w128[:, o:o + mw]

        def w96s(name, i, m0, mw, p, step=64):
            o = W96_OFF[name] + i * step + m0
            return w96[0:p, o:o + mw]

        eye = w128[:, W128_OFF["eye"]:W128_OFF["eye"] + 32]
        wtail = w128[:, W128_OFF["tail"]:W128_OFF["tail"] + 64]

        # ---- SBUF-resident inputs, loaded in fine column chunks in
        # consumption order; mu3's 3 vertically-shifted copies come straight
        # from DRAM via 3-tap strided APs (no SBUF->SBUF shift chain). ----
        xfull = wp.tile([128, NCOL], BF16)
        mu3 = wp.tile([96, NCOL + 2 * ST], BF16)
        # geometric chunks: small leading chunks unblock conv0's first pairs
        # quickly, large tail chunks amortize.  xfull and the mu base row
        # interleave across the sync/scalar HWDGE queues (each ~67 GB/s);
        # mu's two vertically-shifted copies are SBUF->SBUF chunks on the
        # gpsimd SWDGE queue so they cost no sync/scalar bandwidth.
        xc = [0, 700, 1500, 2500, 3700, 5100, 6800, 8800, 11200, 13800, NCOL]
        for k in range(len(xc) - 1):
            a, b = xc[k], xc[k + 1]
            bm = b if k < len(xc) - 2 else NCOL + 2 * ST
            eng = nc.sync if k % 2 == 0 else nc.scalar
            eng2 = nc.scalar if k % 2 == 0 else nc.sync
            eng.dma_start(out=xfull[:, a:b], in_=d_xpad.ap()[:, a:b])
            eng2.dma_start(out=mu3[0:32, a:bm], in_=d_mupad.ap()[:, a:bm])
        for k in range(len(xc) - 1):
            a, b = xc[k], xc[k + 1]
            nc.gpsimd.dma_start(out=mu3[32:64, a:b], in_=mu3[0:32, a + ST:b + ST])
            nc.gpsimd.dma_start(out=mu3[64:96, a:b],
                                in_=mu3[0:32, a + 2 * ST:b + 2 * ST])
        # mid/last conv weights stream behind the input chunks (needed from
        # conv1 onwards, ~60us in)
        nc.sync.dma_start(out=w128[:, 576:W128_COLS],
                          in_=d_w128.ap()[:, 576:W128_COLS])
        nc.scalar.dma_start(out=w96[:, 192:W96_COLS],
                            in_=d_w96.ap()[:, 192:W96_COLS])

        # ---- q tiles ----
        def new_q(tag):
            q = big.tile([128, NCOL], BF16, tag=tag)
            nc.gpsimd.memset(q[0:64, 0:130], 0.0)
            inter = q[0:64, 258:258 + 127 * ST].rearrange(
                "p (m s) -> p m s", s=ST)[:, :, 0:1]
            nc.gpsimd.memset(inter, 0.0)
            nc.gpsimd.memset(q[0:64, ST * 129:NCOL], 0.0)
            up_inter = q[64:128, 0:ST * 128].rearrange(
                "p (m s) -> p m s", s=ST)[:, :, 0:1]
            nc.gpsimd.memset(up_inter, 0.0)
            last_up = _j0(strips[-1][0]) - ST + strips[-1][1] * ST
            nc.gpsimd.memset(q[64:128, last_up:NCOL], 0.0)
            return q

        def view3(ap2d, n):
            return ap2d.rearrange("p (r c) -> p r c", c=ST)[:, :, 0:128]

        def evac_pair(ps, q, pa, pb, bcol):
            # single-op leaky relu: q = lrelu(ps + bias), alpha=0.01
            (r0a, nra) = pa
            j0a = _j0(r0a)
            na = ST * nra
            srcA = view3(ps[0:64, 0:na], na)
            dstA = view3(q[0:64, j0a:j0a + na], na)
            nc.scalar.activation(dstA, srcA, ACTF.Lrelu,
                                 bias=bias[0:64, bcol:bcol + 1], alpha=0.01)
            nc.sync.dma_start(out=q[64:128, j0a - ST:j0a - ST + na],
                              in_=q[0:64, j0a:j0a + na])
            if pb is None:
                return
            (r0b, nrb) = pb
            j0b = _j0(r0b)
            nb = ST * nrb
            srcB = view3(ps[64:128, 0:nb], nb)
            dstB = view3(q[64:128, j0b - ST:j0b - ST + nb], nb)
            nc.scalar.activation(dstB, srcB, ACTF.Lrelu,
                                 bias=bias[64:128, bcol:bcol + 1], alpha=0.01)
            # mirror copies ride sync/gpsimd queues: descriptor generation
            # costs ~600ns of SEQUENCER time, which would starve the evac
            # ACTs if placed on the scalar queue (conv1/2 PE pace is 970ns)
            nc.gpsimd.dma_start(out=q[0:64, j0b:j0b + nb],
                                in_=q[64:128, j0b - ST:j0b - ST + nb])

        TAPS9 = [(a, b) for a in (-1, 0, 1) for b in (-1, 0, 1)]

        # ================= conv0 (direct reads from xfull / mu3) =============
        q1 = new_q("A")
        for (pa, pb) in pairs:
            j0a = _j0(pa[0])
            na = ST * pa[1]
            j0b = _j0(pb[0]) if pb else 0
            nb = ST * pb[1] if pb else 0
            ps = pp.tile([128, 3 * ST], F32, tag="pA")
            for t in range(12):
                first = (t == 0)
                stop = (t == 11)
                if t < 9:
                    dr, dc = TAPS9[t]
                    oA = j0a + ST * dr + dc
                    oB = j0b + ST * dr + dc
                    nc.tensor.matmul(ps[0:64, 0:na], w128s("w0c1", t, 0, 64),
                                     xfull[:, oA:oA + na], start=first, stop=stop)
                    if pb is not None:
                        nc.tensor.matmul(ps[64:128, 0:nb], w128s("w0c1", t, 0, 64),
                                         xfull[:, oB:oB + nb], start=first, stop=stop)
                else:
                    dc = (-1, 0, 1)[t - 9]
                    oA = j0a - ST + dc
                    oB = j0b - ST + dc
                    nc.tensor.matmul(ps[0:64, 0:na], w96s("w0c2", t - 9, 0, 64, 96),
                                     mu3[0:96, oA:oA + na], start=first, stop=stop)
                    if pb is not None:
                        nc.tensor.matmul(ps[64:128, 0:nb], w96s("w0c2", t - 9, 0, 64, 96),
                                         mu3[0:96, oB:oB + nb], start=first, stop=stop)
            evac_pair(ps, q1, pa, pb, 0)
        if debug:
            nc.sync.dma_start(out=d_q1.ap(), in_=q1[:])

        # ================= conv1 / conv2 =================
        def mid_conv(qin, qout, wPname, wSname, bcol):
            for (pa, pb) in pairs:
                j0a = _j0(pa[0])
                na = ST * pa[1]
                j0b = _j0(pb[0]) if pb else 0
                nb = ST * pb[1] if pb else 0
                ps = pp.tile([128, 3 * ST], F32, tag="pA")
                for t in range(6):
                    first = (t == 0)
                    stop = (t == 5)
                    if t < 3:
                        dc = (-1, 0, 1)[t]
                        oA = j0a - ST + dc
                        oB = j0b - ST + dc
                        nc.tensor.matmul(ps[0:64, 0:na], w128s(wPname, t, 0, 64),
                                         qin[0:128, oA:oA + na], start=first, stop=stop)
                        if pb is not None:
                            nc.tensor.matmul(ps[64:128, 0:nb], w128s(wPname, t, 0, 64),
                                             qin[0:128, oB:oB + nb], start=first, stop=stop)
                    else:
                        dc = (-1, 0, 1)[t - 3]
                        oA = j0a + ST + dc
                        oB = j0b + ST + dc
                        nc.tensor.matmul(ps[0:64, 0:na], w96s(wSname, t - 3, 0, 64, 64),
                                         qin[0:64, oA:oA + na], start=first, stop=stop)
                        if pb is not None:
                            nc.tensor.matmul(ps[64:128, 0:nb], w96s(wSname, t - 3, 0, 64, 64),
                                             qin[0:64, oB:oB + nb], start=first, stop=stop)
                evac_pair(ps, qout, pa, pb, bcol)

        q2 = new_q("B")
        mid_conv(q1, q2, "w1P", "w1S", 1)
        if debug:
            nc.sync.dma_start(out=d_q2.ap(), in_=q2[:])

        # ================= conv_last + softmax + fusion =================
        xp_ap = d_xpad.ap()

        def patch_src(tap0, ntap, j0, w):
            # [ntap x 32 x w] from xpad mem_stab rows; taps tap0..tap0+ntap-1
            # have consecutive P_TAPS offsets (stride-1 group)
            return AP(tensor=xp_ap.tensor, offset=96 * XCOL + j0 + P_TAPS[tap0],
                      ap=[[1, ntap], [XCOL, 32], [1, w]])

        blocks = []
        i = 0
        while i < len(strips):
            if i + 1 < len(strips):
                blocks.append((i, i + 1))
                i += 2
            else:
                blocks.append((i, None))
                i += 1

        def load_block(bi):
            # mem_stab unfold patches for a 2-strip window; prefetched 2
            # blocks ahead so the PE never waits on them
            (s0, s1) = blocks[bi]
            nstrips = 1 if s1 is None else 2
            j00 = _j0(strips[s0][0])
            w = sum(ST * strips[s0 + k][1] for k in range(nstrips))
            msa = fu.tile([128, 6 * ST + 2], BF16, tag="msa", bufs=3, name="msa")
            msb = fu.tile([128, 6 * ST + 2], BF16, tag="msb", bufs=3, name="msb")
            msc = fu.tile([128, 6 * ST + 2], BF16, tag="msc", bufs=3, name="msc")
            nc.gpsimd.dma_start(out=msa[0:96, 0:w], in_=patch_src(0, 3, j00, w))
            nc.sync.dma_start(out=msa[96:128, 0:w], in_=patch_src(3, 1, j00, w))
            nc.gpsimd.dma_start(out=msb[0:64, 0:w], in_=patch_src(4, 2, j00, w))
            nc.gpsimd.dma_start(out=msb[64:128, 0:w], in_=patch_src(6, 2, j00, w))
            # tap-8 patch twice, at partition bases 64 and 96, so each strip's
            # product op is same-base with its exp (pc3 col group 2 / 3)
            nc.sync.dma_start(out=msc[64:96, 0:w], in_=patch_src(8, 1, j00, w))
            if nstrips == 2:
                nc.sync.dma_start(out=msc[96:128, 0:w], in_=patch_src(8, 1, j00, w))
            return (msa, msb, msc)

        pending = {}
        for bi in range(min(2, len(blocks))):
            pending[bi] = load_block(bi)

        q3 = new_q("A")
        mid_conv(q2, q3, "w2P", "w2S", 2)
        if debug:
            nc.sync.dma_start(out=d_q3.ap(), in_=q3[:])

        def head12(s):
            # taps 0-7 of the fusion head for one strip: 2 M=128 psum chunks
            (r0, nr) = strips[s]
            j0 = _j0(r0)
            n = ST * nr
            out = []
            for ci in range(2):
                m0 = 128 * ci
                psc = ph.tile([128, 3 * ST], F32, tag="ph", name=f"ph{ci}")
                for i, dc in enumerate((-1, 0, 1)):
                    o = j0 - ST + dc
                    nc.tensor.matmul(psc[:, 0:n], w128s("wlP", i, m0, 128, 288),
                                     q3[0:128, o:o + n], start=(i == 0), stop=False)
                for i, dc in enumerate((-1, 0, 1)):
                    o = j0 + ST + dc
                    nc.tensor.matmul(psc[:, 0:n], w96s("wlS", i, m0, 128, 64, 288),
                                     q3[0:64, o:o + n], start=False, stop=(i == 2))
                out.append(psc)
            return out

        def head8_pair(s0, s1):
            # tap-8 head for both strips, col-tiled to PE col groups 2 / 3
            # (same stationary weights per tap -> the two matmuls overlap)
            pc3 = pcp.tile([128, 3 * ST], F32, tag="pcp")
            js = [_j0(strips[s0][0]), _j0(strips[s1][0]) if s1 is not None else 0]
            ns = [ST * strips[s0][1], ST * strips[s1][1] if s1 is not None else 0]
            for i, dc in enumerate((-1, 0, 1)):
                for k, (j0, n) in enumerate(zip(js, ns)):
                    if k == 1 and s1 is None:
                        continue
                    o = j0 - ST + dc
                    nc.tensor.matmul(pc3[64 + 32 * k:96 + 32 * k, 0:n],
                                     w128s("wlP", i, 256, 32, 288),
                                     q3[0:128, o:o + n], start=(i == 0), stop=False,
                                     tile_position=(0, 64 + 32 * k))
            for i, dc in enumerate((-1, 0, 1)):
                for k, (j0, n) in enumerate(zip(js, ns)):
                    if k == 1 and s1 is None:
                        continue
                    o = j0 + ST + dc
                    nc.tensor.matmul(pc3[64 + 32 * k:96 + 32 * k, 0:n],
                                     w96s("wlS", i, 256, 32, 64, 288),
                                     q3[0:64, o:o + n], start=False, stop=(i == 2),
                                     tile_position=(0, 64 + 32 * k))
            return pc3

        def numden_mms(st):
            # shared-weight rounds, 4-way col-tiled:
            # pnd = [den_s, den_s1, num_s, num_s1] in one psum bank (dens at
            # base 0 so fuse_post's reciprocal runs once at base 0).
            # tail weights fold the softmax "+1" via the ones rows of t8.
            pndt, parts = st[0], st[1]
            mms = []
            for rnd, key in enumerate(("ta", "tb")):
                for k, pt in enumerate(parts):
                    n = pt["n"]
                    mms.append((pndt[64 + 32 * k:96 + 32 * k, 0:n], eye,
                                pt[key][:, 0:n], rnd == 0, False, (0, 64 + 32 * k)))
                    mms.append((pndt[32 * k:32 + 32 * k, 0:n], eye,
                                pt["e" + key[1]][:, 0:n], rnd == 0, False, (0, 32 * k)))
            # tail round: eye[64:128] = [I;I] sums the two partition groups of
            # e8 ([exp8, ones] -> den gets exp8+1) and pr ([prod, z] -> num);
            # all four K=64 matmuls share one lhsT -> 4-way col overlap
            s2 = eye[64:128, :]
            for k, pt in enumerate(parts):
                n = pt["n"]
                lo, hi = 64 + 32 * k, 96 + 32 * k
                mms.append((pndt[32 * k:32 + 32 * k, 0:n], eye[lo:hi, :],
                            pt["e8"][lo:hi, 0:n], False, True, (lo, 32 * k)))
                mms.append((pndt[64 + 32 * k:96 + 32 * k, 0:n], s2,
                            pt["pr"][64:128, 0:n], False, True, (64, 64 + 32 * k)))
            return mms

        def issue(mms):
            for (out, lhsT, rhs, start, stop, tp) in mms:
                nc.tensor.matmul(out, lhsT, rhs, start=start, stop=stop,
                                 tile_position=tp, skip_group_check=True)

        def fuse_post(st):
            # all DVE ops same-base (walrus checkSBSameStartPartition); the
            # den->num partition re-alignment goes through a small DMA copy.
            # reciprocal covers both strips' dens in one base-0 op.
            pndt, parts = st[0], st[1]
            np_ = 32 * len(parts)
            n = max(pt["n"] for pt in parts)
            rd = fu.tile([128, 3 * ST], F32, tag="rd", name="rd")
            ost = fu.tile([128, 3 * ST], F32, tag="ost", name="ost")
            nc.vector.tensor_scalar_add(pndt[0:np_, 0:n], pndt[0:np_, 0:n], 1.0)
            nc.vector.reciprocal_approx_fast(rd[0:np_, 0:n], pndt[0:np_, 0:n])
            nc.gpsimd.dma_start(out=rd[64:64 + np_, 0:n], in_=rd[0:np_, 0:n])
            nc.vector.tensor_tensor(ost[64:64 + np_, 0:n], pndt[64:64 + np_, 0:n],
                                    rd[64:64 + np_, 0:n], op=ALU.mult)
            for k, pt in enumerate(parts):
                (r0, nr) = strips[pt["s"]]
                nst = ST * nr
                src = view3(ost[64 + 32 * k:96 + 32 * k, 0:nst], nst)
                nc.sync.dma_start(out=d_out.ap()[:, r0:r0 + nr, :], in_=src)

        def warm_burst(nmm):
            # dense same-weight dummy matmuls: a >=3.4us bubble-free PE run is
            # the only thing that lifts the HAM clock gate (1.2 -> 2.4 GHz)
            # once a stall has tripped it; the fusion's 6-matmul psum chains
            # are too short to ever re-warm on their own
            wb = pp.tile([128, 512], F32, tag="pA", name="wb")
            for i in range(nmm):
                nc.tensor.matmul(wb[:, 0:512], wz[:, 0:128], wz[:, 0:512],
                                 start=(i == 0), stop=(i == nmm - 1))

        def dbg_dump_pnd(st):
            pndt = st[0]
            tmp = fu.tile([128, 3 * ST], F32, tag="dbgtmp", name="dbgtmp")
            nc.vector.tensor_scalar_add(tmp[:, :], pndt[:, :], 0.0)
            nc.sync.dma_start(out=d_pnd.ap(), in_=tmp[:, :])

        prev = None
        for bi, (s0, s1) in enumerate(blocks):
            nstrips = 1 if s1 is None else 2
            msa, msb, msc = pending.pop(bi)
            if bi + 2 < len(blocks):
                pending[bi + 2] = load_block(bi + 2)
            if bi == 1:
                warm_burst(8)       # cover the fusion pipeline-ramp stall
            elif bi in (4, 9, 14, 19):
                warm_burst(14)      # guaranteed re-warm if throttled

            # head matmuls first; the PREVIOUS block's softmax reduce is
            # issued after them so its operand chain (exp -> mul -> prod)
            # has a full block of slack -> no PE stall at block
            # boundaries (a >1us PE idle re-throttles the HAM clock gate)
            pc3 = head8_pair(s0, s1)
            if debug and bi == DBG_BLOCK:
                tmp3 = fu.tile([128, 3 * ST], F32, tag="dbgtmp3", name="dbgtmp3")
                nc.vector.tensor_scalar_add(tmp3[64:128, :], pc3[64:128, :], 0.0)
                nc.sync.dma_start(out=d_pc3.ap(), in_=tmp3[:, :])
            hp = [head12(s0 + k) for k in range(nstrips)]
            if prev is not None:
                issue(numden_mms(prev))
                if debug and prev[2] == DBG_BLOCK:
                    dbg_dump_pnd(prev)

            # ---- Scalar exps + Vector products ----
            # scalar FIFO: the two e8 exps go FIRST (they free the pcp psum
            # slot for the next block's pc3 chain); vector FIFO: this block's
            # products go before the previous block's fuse_post, so the next
            # numden never waits on a recip stuck ahead in the queue
            # ONE exp covers both strips' tap-8 heads (pc3 col groups 2+3);
            # the "+1" of the softmax denominator is added in fuse_post
            parts = []
            nmax = ST * strips[s0][1]
            e8p = fu.tile([128, 3 * ST], BF16, tag="e8p", name="e8p", bufs=3)
            nc.scalar.activation(e8p[64:64 + 32 * nstrips, 0:nmax],
                                 pc3[64:64 + 32 * nstrips, 0:nmax],
                                 ACTF.Exp, bias=blp[64:64 + 32 * nstrips, 2:3])
            off = 0
            for k in range(nstrips):
                s = s0 + k
                n = ST * strips[s][1]
                parts.append(dict(s=s, n=n, e8=e8p))
            for k in range(nstrips):
                s = s0 + k
                j0 = _j0(strips[s][0])
                n = parts[k]["n"]
                lo, hi = 64 + 32 * k, 96 + 32 * k
                olo, ohi = 96 - 32 * k, 128 - 32 * k
                pr = fu.tile([128, 3 * ST], BF16, tag=f"pr{k}", name="pr", bufs=3)
                ea = fu.tile([128, 3 * ST], BF16, tag=f"ea{k}", name="ea", bufs=3)
                eb = fu.tile([128, 3 * ST], BF16, tag=f"eb{k}", name="eb", bufs=3)
                nc.scalar.activation(ea[:, 0:n], hp[k][0][:, 0:n], ACTF.Exp,
                                     bias=blp[:, 0:1])
                nc.scalar.activation(eb[:, 0:n], hp[k][1][:, 0:n], ACTF.Exp,
                                     bias=blp[:, 1:2])
                ta = fu.tile([128, 3 * ST], BF16, tag=f"ta{k}", name="ta", bufs=3)
                tb = fu.tile([128, 3 * ST], BF16, tag=f"tb{k}", name="tb", bufs=3)
                nc.vector.tensor_mul(pr[lo:hi, 0:n], parts[k]["e8"][lo:hi, 0:n],
                                     msc[lo:hi, off:off + n])
                nc.vector.tensor_mul(ta[:, 0:n], ea[:, 0:n], msa[:, off:off + n])
                nc.vector.tensor_mul(tb[:, 0:n], eb[:, 0:n], msb[:, off:off + n])
                nc.sync.dma_start(out=pr[olo:ohi, 0:n], in_=xfull[64:96, j0:j0 + n])
                if debug and bi == DBG_BLOCK and k == 1:
                    nc.sync.dma_start(out=d_ea1.ap(), in_=ea[:, :])
                    nc.sync.dma_start(out=d_ta1.ap(), in_=ta[:, :])
                parts[k].update(ta=ta, tb=tb, ea=ea, eb=eb, pr=pr)
                off += n
            if prev is not None:
                fuse_post(prev)
            pndt = pnd.tile([128, 3 * ST], F32, tag="pnd")
            prev = (pndt, parts, bi)

        issue(numden_mms(prev))
        fuse_post(prev)

    nc.compile()
    return nc


BF16_NP = mybir.dt.np(mybir.dt.bfloat16)


def _pad_rows(x, cols):
    c = x.shape[0]
    buf = np.zeros((c, cols), dtype=BF16_NP)
    buf[:, 130:130 + ST * 128].reshape(c, 128, ST)[:, :, 0:128] = x.astype(BF16_NP)
    return buf


def _prep_shared(w0, b0, w1, b1, w2, b2, w_last, b_last):
    f = np.float32
    w0t = np.transpose(np.asarray(w0, f), (1, 2, 3, 0))      # [160,3,3,64]
    w0c1 = np.ascontiguousarray(w0t[0:128].reshape(128, 9 * 64))
    w0c2 = np.ascontiguousarray(
        np.transpose(w0t[128:160], (1, 0, 2, 3)).reshape(96, 3 * 64))
    def mid(w):
        wt = np.transpose(np.asarray(w, f), (1, 2, 3, 0))    # [64,3,3,64]
        wP = np.ascontiguousarray(
            np.concatenate([wt[:, 0], wt[:, 1]], 0).reshape(128, 3 * 64))
        wS = np.ascontiguousarray(wt[:, 2].reshape(64, 3 * 64))
        return wP, wS
    w1P, w1S = mid(w1)
    w2P, w2S = mid(w2)
    perm = np.array([(pp % 32) * 9 + pp // 32 for pp in range(288)])
    wl2 = np.asarray(w_last, f)[perm]                        # [288,64,3,3] p-major
    wlt = np.transpose(wl2, (1, 2, 3, 0))                    # [64,3,3,288]
    wlP = np.ascontiguousarray(
        np.concatenate([wlt[:, 0], wlt[:, 1]], 0).reshape(128, 3 * 288))
    wlS = np.ascontiguousarray(wlt[:, 2].reshape(64, 3 * 288))
    eye = np.tile(np.eye(32, dtype=f), (4, 1))
    i32 = np.eye(32, dtype=f)
    tail = np.zeros((128, 64), f)
    tail[64:96, 0:32] = i32    # den: exp(tap8)
    tail[96:128, 0:32] = i32   # den: +1 (ones rows of t8)
    tail[0:32, 32:64] = i32    # num: exp(tap8)*patch8
    tail[32:64, 32:64] = i32   # num: z (logit-0 slot)

    w128 = np.zeros((128, W128_COLS), f)
    w128[:, W128_OFF["w0c1"]:W128_OFF["w0c1"] + 576] = w0c1
    w128[:, W128_OFF["w1P"]:W128_OFF["w1P"] + 192] = w1P
    w128[:, W128_OFF["w2P"]:W128_OFF["w2P"] + 192] = w2P
    w128[:, W128_OFF["wlP"]:W128_OFF["wlP"] + 864] = wlP
    w128[:, W128_OFF["eye"]:W128_OFF["eye"] + 32] = eye
    w128[:, W128_OFF["tail"]:W128_OFF["tail"] + 64] = tail
    w96 = np.zeros((96, W96_COLS), f)
    w96[0:96, W96_OFF["w0c2"]:W96_OFF["w0c2"] + 192] = w0c2
    w96[0:64, W96_OFF["w1S"]:W96_OFF["w1S"] + 192] = w1S
    w96[0:64, W96_OFF["w2S"]:W96_OFF["w2S"] + 192] = w2S
    w96[0:64, W96_OFF["wlS"]:W96_OFF["wlS"] + 864] = wlS

    b3 = np.stack([np.asarray(b0, f), np.asarray(b1, f), np.asarray(b2, f)],
                  axis=1)                                    # [64, 3]
    bias = np.concatenate([b3, b3], axis=0)                  # [128, 3]
    blp_flat = np.asarray(b_last, f)[perm]
    blp = np.zeros((128, 3), f)
    blp[:, 0] = blp_flat[0:128]
    blp[:, 1] = blp_flat[128:256]
    blp[64:96, 2] = blp_flat[256:288]
    blp[96:128, 2] = blp_flat[256:288]
    out = dict(w128=w128.astype(BF16_NP), w96=w96.astype(BF16_NP),
               bias=np.ascontiguousarray(bias), blp=blp)
    return out


_NC_CACHE = {}


def _get_nc(debug=False):
    if debug not in _NC_CACHE:
        _NC_CACHE[debug] = _build_program(debug)
    return _NC_CACHE[debug]


def make_in_maps(z, backbone, mem_stab, mem_unstab, shared):
    f = np.float32
    z = np.asarray(z, f); backbone = np.asarray(backbone, f)
    ms = np.asarray(mem_stab, f); mu = np.asarray(mem_unstab, f)
    maps = []
    for b in range(z.shape[0]):
        x160 = np.concatenate([backbone[b], z[b], ms[b]], axis=0)  # [128,...]
        maps.append(dict(xpad=_pad_rows(x160, XCOL),
                         mupad=_pad_rows(mu[b], MUCOL), **shared))
    return maps


def kernel(z, backbone, mem_stab, mem_unstab, w0, b0, w1, b1, w2, b2,
           w_last, b_last, fusion_kernel_size):
    assert int(fusion_kernel_size) == 3
    shared = _prep_shared(w0, b0, w1, b1, w2, b2, w_last, b_last)
    in_maps = make_in_maps(z, backbone, mem_stab, mem_unstab, shared)
    nc = _get_nc()
    res = run_bass_kernel_spmd(nc, in_maps, core_ids=list(range(len(in_maps))))
    out = np.stack([r["out"] for r in res.results], axis=0)
    return out.astype(np.float32)
